# revision 18
# baseline (speedup 1.0000x reference)
# Trainium2 Bass kernel for nn_MultiHeadAttention_24902220382931.
#
# Strategy: data-parallel over sentences. The 32 variable-length sentences are
# sorted by length; core c processes ranks {c, 15-c, 16+c, 31-c} (exactly equal
# token counts, near-equal attention work). Each core packs its 4 sentences
# into 4 fixed-size slots (max length per slot across cores, regions rounded to
# 128) so that all 8 cores execute one identical SPMD program. Padded rows are
# zeros; softmax denominators are corrected by subtracting the per-core pad
# count (pad keys contribute exp(0)=1 exactly), shipped as data.
#
# Precision: matmul operands in bf16 (full PE rate; fp32 matmul is 4x slower),
# accumulation in fp32 PSUM, softmax sum / residual / layernorm in fp32.
import sys

for _p in ("/opt/trn_rl_repo", "/root/.axon_site/_ro/trn_rl_repo"):
    if _p not in sys.path:
        sys.path.insert(0, _p)

import numpy as np
import ml_dtypes

import concourse.bass as bass  # noqa: F401  (bass types used via bacc/tile)
import concourse.mybir as mybir
import concourse.tile as tile
from concourse import bacc

BF16 = ml_dtypes.bfloat16
F32 = np.float32

N_CORES = 8
MB = 32
D_MODEL = 1024
D_HALF = 512  # d_content == d_pos
N_HEAD = 8
D_K = 128
DK2 = 64
SCALE = float(D_MODEL) ** 0.5  # 32.0
EPS = 1e-3
P = 128  # partitions


def _ceil_to(x, m):
    return (x + m - 1) // m * m


class Plan:
    def __init__(self, lengths):
        lengths = np.asarray(lengths, np.int64)
        assert lengths.shape == (MB,)
        order = np.argsort(-lengths, kind="stable")
        # core c handles sentence ranks {c, 15-c, 16+c, 31-c} (desc length order)
        self.core_sents = [
            [int(order[c]), int(order[15 - c]), int(order[16 + c]), int(order[31 - c])]
            for c in range(N_CORES)
        ]
        self.lengths = lengths
        self.slot_pad = [
            max(int(lengths[self.core_sents[c][j]]) for c in range(N_CORES))
            for j in range(4)
        ]
        self.regions = [_ceil_to(sp, P) for sp in self.slot_pad]
        self.offs = [0]
        for r in self.regions[:-1]:
            self.offs.append(self.offs[-1] + r)
        self.t_pad = sum(self.regions)
        assert self.t_pad % P == 0
        self.nt = self.t_pad // P
        self.glob_off = np.concatenate([[0], np.cumsum(lengths)[:-1]]).astype(np.int64)

    @property
    def key(self):
        return (tuple(self.slot_pad), self.t_pad)



def _copy(nc, eng, out, in_):
    # engine-dispatched copy: DVE has tensor_copy, ACT uses activation(Copy)
    if eng is nc.scalar:
        nc.scalar.copy(out, in_)
    else:
        eng.tensor_copy(out, in_)


def _transpose_qi(nc, at_ps, entry, L, nk):
    # attn^T @ diag(recip): transpose + normalize in one matmul per k-chunk
    attn, diag, lq, qoff = entry
    for ki in range(nk):
        kc = min(P, L - P * ki)
        nc.tensor.matmul(
            at_ps[ki][0:kc, qoff:qoff + lq],
            attn[0:lq, P * ki:P * ki + kc],
            diag[0:lq, 0:lq],
            start=True,
            stop=True,
        )

def _build_program(plan: Plan, loop_n: int = 1):
    """Build and compile the single-core Bass program (same for all cores).

    loop_n > 1 wraps the whole computation in a hardware For-loop (for
    steady-state timing measurements; the body is idempotent)."""
    import contextlib
    T = plan.t_pad
    NT = plan.nt
    nc = bacc.Bacc("TRN2", target_bir_lowering=False, debug=False)

    dt = mybir.dt
    # ---- DRAM I/O ----
    # xT packed per-slot: [p, s*(8*gw)] with [p, c, t] = x-dim c*128+p of
    # token t -> one contiguous 8KB descriptor per partition per slot
    xT_d = nc.dram_tensor("xT", [P, 8 * T], dt.bfloat16, kind="ExternalInput").ap()
    x_d = nc.dram_tensor("x", [T, D_MODEL], dt.float32, kind="ExternalInput").ap()
    # weights packed partition-major on host: one contiguous DMA each
    wq_d = nc.dram_tensor("wq", [P, 4 * 8 * P], dt.bfloat16, kind="ExternalInput").ap()
    wk_d = nc.dram_tensor("wk", [P, 4 * 8 * P], dt.bfloat16, kind="ExternalInput").ap()
    wv_d = nc.dram_tensor("wv", [P, 8 * D_HALF], dt.bfloat16, kind="ExternalInput").ap()
    pw_d = nc.dram_tensor("pw", [P, 8 * D_HALF], dt.bfloat16, kind="ExternalInput").ap()
    npad_d = nc.dram_tensor("npad", [P, 4], dt.float32, kind="ExternalInput").ap()
    ident_d = nc.dram_tensor("ident", [P, P], dt.bfloat16, kind="ExternalInput").ap()
    out_d = nc.dram_tensor("out", [T, D_MODEL], dt.float32, kind="ExternalOutput").ap()

    with tile.TileContext(nc) as tc:
        with (
            tc.tile_pool(name="persist", bufs=1) as pp,
            tc.tile_pool(name="weights", bufs=1) as wp,
        ):
            # Per-slot persistent tensors (finer dependency granularity lets
            # attention/proj start as soon as a slot's QKV is done).
            # Q^T/K^T pair-stacked: [p, comp(c/p), pair, region]; partition
            # p<64 holds head 2*pair, p>=64 head 2*pair+1 (comp's 64 dims).
            qt = [pp.tile([P, 2, 4, r], dt.bfloat16, name=f"qt{s}", tag=f"qt{s}")
                  for s, r in enumerate(plan.regions)]
            kt = [pp.tile([P, 2, 4, r], dt.bfloat16, name=f"kt{s}", tag=f"kt{s}")
                  for s, r in enumerate(plan.regions)]
            # V token-natural, head-major columns: [p, tile, head, {c64|p64}]
            vv = [pp.tile([P, r // P, D_MODEL], dt.bfloat16, name=f"vv{s}", tag=f"vv{s}")
                  for s, r in enumerate(plan.regions)]
            o1t = [pp.tile([P, 4, r], dt.bfloat16, name=f"o1t{s}", tag=f"o1t{s}")
                   for s, r in enumerate(plan.regions)]
            o2t = [pp.tile([P, 4, r], dt.bfloat16, name=f"o2t{s}", tag=f"o2t{s}")
                   for s, r in enumerate(plan.regions)]
            npad_sb = pp.tile([P, 4], dt.float32, tag="npad")
            ident_sb = pp.tile([P, P], dt.bfloat16, tag="ident")

            wq_sb = wp.tile([P, 4, 8, P], dt.bfloat16, tag="wq")
            wk_sb = wp.tile([P, 4, 8, P], dt.bfloat16, tag="wk")
            wv_sb = wp.tile([P, 8, D_HALF], dt.bfloat16, tag="wv")
            pw_sb = wp.tile([P, 2, 4, D_HALF], dt.bfloat16, tag="pw")

            _c = getattr(plan, "cfg", {})
            aux_eng = getattr(nc, _c.get("aux_eng", "gpsimd"))
            w_eng = getattr(nc, _c.get("w_eng", "scalar"))
            aux_eng.dma_start(npad_sb[:, :], npad_d[:, :])
            aux_eng.dma_start(ident_sb[:, :], ident_d[:, :])
            # per-pr-chunk weight DMAs on separate queues: the first QK
            # matmuls are gated on a 256KB chunk, not the full megabyte
            wq_f = wq_sb.rearrange("p a b c -> p a (b c)")
            wk_f = wk_sb.rearrange("p a b c -> p a (b c)")
            wq_r = wq_d.rearrange("p (a r) -> p a r", a=4)
            wk_r = wk_d.rearrange("p (a r) -> p a r", a=4)
            for pr in range(4):
                w_eng.dma_start(wq_f[:, pr, :], wq_r[:, pr, :])
                w_eng.dma_start(wk_f[:, pr, :], wk_r[:, pr, :])
            aux_eng.dma_start(
                wv_sb.rearrange("p a b -> p (a b)")[:, :], wv_d[:, :])
            aux_eng.dma_start(
                pw_sb.rearrange("p a b c -> p (a b c)")[:, :], pw_d[:, :])

            loop_cm = (tc.For_i(0, loop_n, 1,
                                hint_engines=(mybir.EngineType.PE,
                                              mybir.EngineType.DVE,
                                              mybir.EngineType.Activation,
                                              mybir.EngineType.SP))
                       if loop_n > 1 else contextlib.nullcontext())
            with loop_cm:
                _kernel_body(nc, tc, plan, locals())

    nc.compile()
    return nc


def _kernel_body(nc, tc, plan, env):
    dt = mybir.dt
    qt, kt, vv, o1t, o2t = (env["qt"], env["kt"], env["vv"], env["o1t"],
                            env["o2t"])
    npad_sb, ident_sb = env["npad_sb"], env["ident_sb"]
    wq_sb, wk_sb, wv_sb, pw_sb = (env["wq_sb"], env["wk_sb"], env["wv_sb"],
                                  env["pw_sb"])
    xT_d, x_d, out_d = env["xT_d"], env["x_d"], env["out_d"]
    cfg = getattr(plan, "cfg", dict(qk=4, v=3, lg=3, at=1, ot=1, z=6))
    out_eng = getattr(nc, cfg.get("out_eng", "gpsimd"))
    aux_eng = getattr(nc, cfg.get("aux_eng", "gpsimd"))
    w_eng = getattr(nc, cfg.get("w_eng", "scalar"))
    ms_gp = cfg.get("ms_gp", True)
    ln_gp = cfg.get("ln_gp", True)

    # zero the attention-output staging (pad-query columns are never
    # written; keep them finite for the projection matmuls)
    # only the pad columns [L, region) are never written by attention
    ms_eng = nc.gpsimd if ms_gp else nc.vector
    for s in range(4):
        L, r = plan.slot_pad[s], plan.regions[s]
        if L < r:
            ms_eng.memset(o1t[s][:, :, L:r], 0.0)
            ms_eng.memset(o2t[s][:, :, L:r], 0.0)

    # ================= Phase 1: QKV projections =================
    with (
        tc.tile_pool(name="xt_pool", bufs=2) as xtp,
        tc.tile_pool(name="qk_ps", bufs=cfg["qk"], space="PSUM") as qkps,
        tc.tile_pool(name="v_ps", bufs=cfg["v"], space="PSUM") as vps,
    ):
        for s in range(4):
            gw = plan.regions[s]
            g0 = plan.offs[s]
            xt_sb = xtp.tile([P, 8, gw], dt.bfloat16, tag="xt")
            nc.sync.dma_start(
                xt_sb.rearrange("p a b -> p (a b)")[:, :],
                xT_d[:, 8 * g0:8 * (g0 + gw)])
            for pr in range(4):
                for qk, (w_sb, dst) in enumerate(((wq_sb, qt), (wk_sb, kt))):
                    for half in range(2):  # 0: content dims, 1: pos dims
                        acc = qkps.tile([P, 512], dt.float32, tag="qkacc")
                        for jj in range(4):
                            j = half * 4 + jj
                            nc.tensor.matmul(
                                acc[:, 0:gw],
                                w_sb[:, pr, j, :],
                                xt_sb[:, j, 0:gw],
                                start=(jj == 0),
                                stop=(jj == 3),
                            )
                        eng = nc.vector if (pr + qk + half) % 2 else nc.scalar
                        _copy(nc, eng, dst[s][:, half, pr, 0:gw], acc[:, 0:gw])
            vv_w = vv[s].rearrange("p t (h b d) -> p t h b d", h=N_HEAD, b=2)
            for tt in range(gw // P):
                tl = tt * P
                for half in range(2):
                    vacc = vps.tile([P, 512], dt.float32, tag="vacc")
                    vacc_r = vacc.rearrange("p (h d) -> p h d", h=N_HEAD)
                    for jj in range(4):
                        j = half * 4 + jj
                        nc.tensor.matmul(
                            vacc[:, :],
                            xt_sb[:, j, tl:tl + P],
                            wv_sb[:, j, :],
                            start=(jj == 0),
                            stop=(jj == 3),
                        )
                    # scatter head h's 64 cols to h*128 + half*64
                    nc.scalar.copy(vv_w[:, tt, :, half, :], vacc_r[:, :, :])

    # ======== Phase 2+3: attention + proj/LN interleaved per slot ========
    # PSUM: lg(2) + atp0..3(4) + ozp(2, shared by attn-out and proj acc) = 8
    x_dma = nc.sync
    out_dma = out_eng
    with (
        tc.tile_pool(name="lg_ps", bufs=2, space="PSUM") as lgps,
        tc.tile_pool(name="at_ps", bufs=1, space="PSUM") as atps,
        tc.tile_pool(name="ozp_ps", bufs=2, space="PSUM") as ozps,
        tc.tile_pool(name="attn_sb", bufs=4) as asb,
        tc.tile_pool(name="small_sb", bufs=6) as ssb,
        tc.tile_pool(name="z_sb", bufs=3) as zsb,
        tc.tile_pool(name="x_sb", bufs=3) as xsb,
        tc.tile_pool(name="ln_sb", bufs=4) as lsb,
    ):
        for s in range(4):
            L = plan.slot_pad[s]
            nk = (L + P - 1) // P
            nq = nk
            vv_r = vv[s].rearrange("p t (h d) -> p t h d", h=N_HEAD)

            def _stage_prev(prev):
                # stage prev head's normalized attn^T slabs PSUM->SBUF,
                # spread over DVE/GpSimd (ACT is exp-bound)
                pot, pat, ph = prev
                sbs = []
                for ki in range(nk):
                    kc = min(P, L - P * ki)
                    at_sb = asb.tile([P, 512], dt.bfloat16, tag="at_sb")
                    eng = nc.vector if (ki % 2 == 0) else nc.scalar
                    _copy(nc, eng, at_sb[0:kc, 0:L], pat[ki][0:kc, 0:L])
                    sbs.append((at_sb, kc))
                return sbs

            def _prev_ot(prev, sbs, ki):
                # one attn@V chunk of the prev head, streamed under the
                # current head's logits
                pot, pat, ph = prev
                sb, kc = sbs[ki]
                nc.tensor.matmul(
                    pot[:, 0:L],
                    vv_r[0:kc, ki, ph, :],
                    sb[0:kc, 0:L],
                    start=(ki == 0),
                    stop=(ki == nk - 1),
                )

            def _prev_out(prev):
                pot, pat, ph = prev
                php, ppr = 64 * (ph % 2), ph // 2
                nc.vector.tensor_copy(
                    o1t[s][php:php + 64, ppr, 0:L], pot[0:64, 0:L])
                nc.vector.tensor_copy(
                    o2t[s][php:php + 64, ppr, 0:L], pot[64:128, 0:L])

            prev = None
            for h in range(N_HEAD):
                half, pr = h % 2, h // 2
                hp = 64 * half
                # attn^T slabs per key-chunk: [kc, all slot queries] so the
                # attn@V contraction runs one N=L matmul per chunk
                at_ps = [atps.tile([P, 512], dt.float32, name=f"atp{ki}",
                                   tag=f"atp{ki}") for ki in range(nk)]
                ot = ozps.tile([P, 512], dt.float32, name="ot", tag="ozp")
                sbs = _stage_prev(prev) if prev is not None else None
                pend = {}
                for qi in range(nq):
                    qoff = P * qi
                    lq = min(P, L - P * qi)
                    lg = lgps.tile([P, 512], dt.float32, tag="lg")
                    for comp in range(2):
                        nc.tensor.matmul(
                            lg[0:lq, 0:L],
                            qt[s][hp:hp + 64, comp, pr, qoff:qoff + lq],
                            kt[s][hp:hp + 64, comp, pr, 0:L],
                            start=(comp == 0),
                            stop=(comp == 1),
                        )
                    if prev is not None and qi < nk:
                        _prev_ot(prev, sbs, qi)
                    attn = asb.tile([P, 512], dt.bfloat16, tag="attn")
                    se = ssb.tile([P, 1], dt.float32, tag="se")
                    nc.scalar.activation(
                        attn[0:lq, 0:L],
                        lg[0:lq, 0:L],
                        mybir.ActivationFunctionType.Exp,
                        scale=1.0 / SCALE,
                        accum_out=se[0:lq, :],
                    )
                    rc = ssb.tile([P, 1], dt.float32, tag="rc")
                    nc.vector.tensor_tensor(
                        rc[0:lq, :], se[0:lq, :], npad_sb[0:lq, s:s + 1],
                        mybir.AluOpType.subtract,
                    )
                    nc.vector.reciprocal(rc[0:lq, :], rc[0:lq, :])
                    diag = ssb.tile([P, P], dt.bfloat16, tag="diag")
                    nc.gpsimd.tensor_scalar(
                        diag[0:lq, 0:lq], ident_sb[0:lq, 0:lq],
                        rc[0:lq, :], None, mybir.AluOpType.mult,
                    )
                    # previous qi's transpose runs after this qi's logits so
                    # exp/diag have a full lg of slack
                    if qi >= 1:
                        _transpose_qi(nc, at_ps, pend[qi - 1], L, nk)
                    pend[qi] = (attn, diag, lq, qoff)
                # flush: last transpose, then prev head's output copies
                _transpose_qi(nc, at_ps, pend[nq - 1], L, nk)
                if prev is not None:
                    _prev_out(prev)
                prev = (ot, at_ps, h)
            # last head of the slot: stage + drain immediately
            sbs = _stage_prev(prev)
            for ki in range(nk):
                _prev_ot(prev, sbs, ki)
            _prev_out(prev)
            # ---- proj + residual + LN for this slot's tiles ----
            gw = plan.regions[s]
            g0 = plan.offs[s]
            for tt in range(gw // P):
                lt = tt * P
                t0 = g0 + lt
                zh = []
                for i, osrc in enumerate((o1t[s], o2t[s])):
                    zp = ozps.tile([P, 512], dt.float32, name="zp", tag="ozp")
                    for k in range(4):
                        nc.tensor.matmul(
                            zp[:, :],
                            osrc[:, k, lt:lt + P],
                            pw_sb[:, i, k, :],
                            start=(k == 0),
                            stop=(k == 3),
                        )
                    zh.append(zp)
                xt_f = xsb.tile([P, D_MODEL], dt.float32, tag="xf")
                x_dma.dma_start(xt_f[:, :], x_d[t0:t0 + P, :])
                z = zsb.tile([P, D_MODEL], dt.float32, tag="z")
                zs0 = lsb.tile([P, 1], dt.float32, tag="zs0")
                zsum = lsb.tile([P, 1], dt.float32, tag="zsum")
                ssq = lsb.tile([P, 1], dt.float32, tag="ssq")
                for i in range(2):
                    nc.vector.tensor_tensor(
                        z[:, i * D_HALF:(i + 1) * D_HALF],
                        zh[i][:, :],
                        xt_f[:, i * D_HALF:(i + 1) * D_HALF],
                        mybir.AluOpType.add,
                    )
                nc.vector.reduce_sum(
                    zsum[:, :], z[:, :], axis=mybir.AxisListType.X
                )
                nc.scalar.activation(
                    xt_f[:, :], z[:, :],
                    mybir.ActivationFunctionType.Square,
                    accum_out=ssq[:, :],
                )
                mu = lsb.tile([P, 1], dt.float32, tag="mu")
                nc.vector.tensor_scalar(
                    mu[:, :], zsum[:, :], 1.0 / D_MODEL, None,
                    mybir.AluOpType.mult,
                )
                var = lsb.tile([P, 1], dt.float32, tag="var")
                nc.vector.tensor_tensor(
                    var[:, :], zsum[:, :], mu[:, :], mybir.AluOpType.mult
                )
                nc.vector.tensor_tensor(
                    var[:, :], ssq[:, :], var[:, :], mybir.AluOpType.subtract
                )
                sig = lsb.tile([P, 1], dt.float32, tag="sig")
                nc.scalar.activation(
                    sig[:, :], var[:, :], mybir.ActivationFunctionType.Sqrt,
                    scale=1.0 / (D_MODEL - 1),
                )
                nc.vector.tensor_scalar(
                    sig[:, :], sig[:, :], EPS, None, mybir.AluOpType.add
                )
                rstd = lsb.tile([P, 1], dt.float32, tag="rstd")
                nc.vector.reciprocal(rstd[:, :], sig[:, :])
                negmu = lsb.tile([P, 1], dt.float32, tag="negmu")
                nc.vector.tensor_scalar(
                    negmu[:, :], zsum[:, :], -1.0 / D_MODEL, None,
                    mybir.AluOpType.mult,
                )
                o = zsb.tile([P, D_MODEL], dt.float32, tag="o")
                eng_o = nc.gpsimd if (ln_gp and tt % 2 == 0) else nc.vector
                eng_o.tensor_scalar(
                    o[:, :], z[:, :], negmu[:, :], rstd[:, :],
                    mybir.AluOpType.add, mybir.AluOpType.mult,
                )
                out_dma.dma_start(out_d[t0:t0 + P, :], o[:, :])


_PROGRAMS = {}   # plan.key -> (nc, plan)
_RUNNERS = {}    # plan.key -> callable(in_maps) -> list[dict]


def _get_program(plan: Plan):
    if plan.key not in _PROGRAMS:
        _PROGRAMS[plan.key] = _build_program(plan)
    return _PROGRAMS[plan.key]


def _make_runner(nc, donate=True):
    """Cached PJRT runner (mirrors bass_utils.run_bass_kernel_spmd's axon
    path via bass2jax, but reuses the jitted executable across calls)."""
    import jax
    from jax.sharding import Mesh, PartitionSpec
    from jax.experimental.shard_map import shard_map
    from concourse import bass2jax

    bass2jax.install_neuronx_cc_hook()

    partition_name = (nc.partition_id_tensor.name
                      if nc.partition_id_tensor else None)
    in_names, out_names, out_avals, zero_shapes = [], [], [], []
    for alloc in nc.m.functions[0].allocations:
        if not isinstance(alloc, mybir.MemoryLocationSet):
            continue
        name = alloc.memorylocations[0].name
        if alloc.kind == "ExternalInput":
            if name == partition_name:
                continue
            in_names.append(name)
        elif alloc.kind == "ExternalOutput":
            out_names.append(name)
            shape = tuple(alloc.tensor_shape)
            dtype = mybir.dt.np(alloc.dtype)
            out_avals.append(jax.core.ShapedArray(shape, dtype))
            zero_shapes.append((shape, dtype))
    n_params = len(in_names)
    all_names = in_names + out_names
    if partition_name is not None:
        all_names = all_names + [partition_name]

    def _body(*args):
        operands = list(args)
        if partition_name is not None:
            operands.append(bass2jax.partition_id_tensor())
        outs = bass2jax._bass_exec_p.bind(
            *operands,
            out_avals=tuple(out_avals),
            in_names=tuple(all_names),
            out_names=tuple(out_names),
            lowering_input_output_aliases=(),
            sim_require_finite=True,
            sim_require_nnan=True,
            nc=nc,
        )
        return tuple(outs)

    devices = jax.devices()[:N_CORES]
    mesh = Mesh(np.asarray(devices), ("core",))
    in_specs = (PartitionSpec("core"),) * (n_params + len(out_names))
    out_specs = (PartitionSpec("core"),) * len(out_names)
    sharded = jax.jit(
        shard_map(_body, mesh=mesh, in_specs=in_specs, out_specs=out_specs,
                  check_rep=False),
        donate_argnums=tuple(range(n_params, n_params + len(out_names)))
        if donate else (),
        keep_unused=True,
    )

    def run(in_maps):
        concat_in = [
            np.concatenate([np.asarray(m[name]) for m in in_maps], axis=0)
            for name in in_names
        ]
        concat_zeros = [
            np.zeros((N_CORES * s[0], *s[1:]), d) for (s, d) in zero_shapes
        ]
        out_arrs = sharded(*concat_in, *concat_zeros)
        return [
            {
                name: np.asarray(out_arrs[i]).reshape(
                    N_CORES, *out_avals[i].shape)[c]
                for i, name in enumerate(out_names)
            }
            for c in range(N_CORES)
        ]

    run.sharded = sharded
    run.in_names = in_names
    run.out_names = out_names
    run.out_avals = out_avals
    run.zero_shapes = zero_shapes
    run.n_params = n_params
    return run


def _prep_weights(w_qs1, w_ks1, w_vs1, w_qs2, w_ks2, w_vs2, proj1_w, proj2_w):
    wq, wk, wv, pw = _prep_weights_4d(w_qs1, w_ks1, w_vs1, w_qs2, w_ks2,
                                      w_vs2, proj1_w, proj2_w)
    # partition-major packing: one contiguous DMA per weight tensor on device
    wq = np.ascontiguousarray(wq.transpose(2, 0, 1, 3).reshape(P, -1))
    wk = np.ascontiguousarray(wk.transpose(2, 0, 1, 3).reshape(P, -1))
    wv = np.ascontiguousarray(wv.transpose(1, 0, 2).reshape(P, -1))
    pw = np.ascontiguousarray(pw.transpose(2, 0, 1, 3).reshape(P, -1))
    return wq, wk, wv, pw


def _prep_weights_4d(w_qs1, w_ks1, w_vs1, w_qs2, w_ks2, w_vs2, proj1_w, proj2_w):
    wq = np.zeros((4, 8, P, P), BF16)
    wk = np.zeros((4, 8, P, P), BF16)
    for pr in range(4):
        h0, h1 = 2 * pr, 2 * pr + 1
        for j in range(8):
            if j < 4:
                rows = slice(j * P, (j + 1) * P)
                wq[pr, j] = np.concatenate(
                    [w_qs1[h0, rows, :], w_qs1[h1, rows, :]], axis=1).astype(BF16)
                wk[pr, j] = np.concatenate(
                    [w_ks1[h0, rows, :], w_ks1[h1, rows, :]], axis=1).astype(BF16)
            else:
                rows = slice((j - 4) * P, (j - 3) * P)
                wq[pr, j] = np.concatenate(
                    [w_qs2[h0, rows, :], w_qs2[h1, rows, :]], axis=1).astype(BF16)
                wk[pr, j] = np.concatenate(
                    [w_ks2[h0, rows, :], w_ks2[h1, rows, :]], axis=1).astype(BF16)
    wv = np.zeros((8, P, D_HALF), BF16)
    for j in range(8):
        src = w_vs1 if j < 4 else w_vs2
        rows = slice((j % 4) * P, (j % 4 + 1) * P)
        wv[j] = np.concatenate([src[h, rows, :] for h in range(8)], axis=1
                               ).astype(BF16)
    pw = np.zeros((2, 4, P, D_HALF), BF16)
    p1T = np.ascontiguousarray(proj1_w.T)  # [in, out]
    p2T = np.ascontiguousarray(proj2_w.T)
    for k in range(4):
        pw[0, k] = p1T[k * P:(k + 1) * P, :].astype(BF16)
        pw[1, k] = p2T[k * P:(k + 1) * P, :].astype(BF16)
    return wq, wk, wv, pw


def _prep_core_inputs(plan: Plan, inp, c):
    T = plan.t_pad
    x = np.zeros((T, D_MODEL), F32)
    npad = np.zeros((4,), F32)
    for j in range(4):
        s = plan.core_sents[c][j]
        L = int(plan.lengths[s])
        g0 = int(plan.glob_off[s])
        x[plan.offs[j]:plan.offs[j] + L] = inp[g0:g0 + L]
        npad[j] = plan.slot_pad[j] - L
    # per-slot packed transpose: [p, c, t] = x[t, c*128+p], slots contiguous
    xT = np.zeros((P, 8 * T), BF16)
    for j in range(4):
        gw, g0 = plan.regions[j], plan.offs[j]
        blk = x[g0:g0 + gw].T.reshape(8, P, gw).transpose(1, 0, 2)
        xT[:, 8 * g0:8 * (g0 + gw)] = blk.reshape(P, 8 * gw).astype(BF16)
    npad_rep = np.tile(npad[None, :], (P, 1)).astype(F32)
    return x, xT, npad_rep


def make_in_maps(plan: Plan, inp, weights):
    wq, wk, wv, pw = weights
    ident = np.eye(P, dtype=BF16)
    in_maps = []
    for c in range(N_CORES):
        x, xT, npad_rep = _prep_core_inputs(plan, inp, c)
        in_maps.append({
            "xT": xT, "x": x, "wq": wq, "wk": wk, "wv": wv, "pw": pw,
            "npad": npad_rep, "ident": ident,
        })
    return in_maps


def gather_output(plan: Plan, results, a_2=None, b_2=None):
    T_tot = int(plan.lengths.sum())
    out = np.empty((T_tot, D_MODEL), F32)
    for c in range(N_CORES):
        oc = results[c]["out"]
        for j in range(4):
            s = plan.core_sents[c][j]
            L = int(plan.lengths[s])
            g0 = int(plan.glob_off[s])
            out[g0:g0 + L] = oc[plan.offs[j]:plan.offs[j] + L]
    if a_2 is not None and (np.any(a_2 != 1.0) or np.any(b_2 != 0.0)):
        out = out * np.asarray(a_2, F32) + np.asarray(b_2, F32)
    return out


def kernel(inp, w_qs1, w_ks1, w_vs1, w_qs2, w_ks2, w_vs2,
           proj1_w, proj2_w, a_2, b_2, token_batch, token_pos, valid_mask):
    inp = np.asarray(inp, F32)
    token_batch = np.asarray(token_batch)
    lengths = np.bincount(token_batch, minlength=MB).astype(np.int64)
    # tokens of each sentence must be contiguous and in order
    plan = Plan(lengths)

    nc = _get_program(plan)
    if plan.key not in _RUNNERS:
        _RUNNERS[plan.key] = _make_runner(nc)
    runner = _RUNNERS[plan.key]

    weights = _prep_weights(np.asarray(w_qs1), np.asarray(w_ks1),
                            np.asarray(w_vs1), np.asarray(w_qs2),
                            np.asarray(w_ks2), np.asarray(w_vs2),
                            np.asarray(proj1_w), np.asarray(proj2_w))
    in_maps = make_in_maps(plan, inp, weights)
    results = runner(in_maps)
    return gather_output(plan, results, np.asarray(a_2), np.asarray(b_2))



# revision 22
# speedup vs baseline: 1.3176x; 1.3176x over previous
# Trainium2 Bass kernel for nn_MultiHeadAttention_24902220382931.
#
# Strategy: data-parallel over sentences. The 32 variable-length sentences are
# sorted by length; core c processes ranks {c, 15-c, 16+c, 31-c} (exactly equal
# token counts, near-equal attention work). Each core packs its 4 sentences
# into 4 fixed-size slots (max length per slot across cores, regions rounded to
# 128) so that all 8 cores execute one identical SPMD program. Padded rows are
# zeros; softmax denominators are corrected by subtracting the per-core pad
# count (pad keys contribute exp(0)=1 exactly), shipped as data.
#
# Precision: matmul operands in bf16 (full PE rate; fp32 matmul is 4x slower),
# accumulation in fp32 PSUM, softmax sum / residual / layernorm in fp32.
import sys

for _p in ("/opt/trn_rl_repo", "/root/.axon_site/_ro/trn_rl_repo"):
    if _p not in sys.path:
        sys.path.insert(0, _p)

import numpy as np
import ml_dtypes

import concourse.bass as bass  # noqa: F401  (bass types used via bacc/tile)
import concourse.mybir as mybir
import concourse.tile as tile
from concourse import bacc

BF16 = ml_dtypes.bfloat16
F32 = np.float32

N_CORES = 8
MB = 32
D_MODEL = 1024
D_HALF = 512  # d_content == d_pos
N_HEAD = 8
D_K = 128
DK2 = 64
SCALE = float(D_MODEL) ** 0.5  # 32.0
EPS = 1e-3
P = 128  # partitions


def _ceil_to(x, m):
    return (x + m - 1) // m * m


class Plan:
    def __init__(self, lengths):
        lengths = np.asarray(lengths, np.int64)
        assert lengths.shape == (MB,)
        order = np.argsort(-lengths, kind="stable")
        # core c handles sentence ranks {c, 15-c, 16+c, 31-c} (desc length order)
        self.core_sents = [
            [int(order[c]), int(order[15 - c]), int(order[16 + c]), int(order[31 - c])]
            for c in range(N_CORES)
        ]
        self.lengths = lengths
        self.slot_pad = [
            max(int(lengths[self.core_sents[c][j]]) for c in range(N_CORES))
            for j in range(4)
        ]
        self.regions = [_ceil_to(sp, P) for sp in self.slot_pad]
        self.offs = [0]
        for r in self.regions[:-1]:
            self.offs.append(self.offs[-1] + r)
        self.t_pad = sum(self.regions)
        assert self.t_pad % P == 0
        self.nt = self.t_pad // P
        self.glob_off = np.concatenate([[0], np.cumsum(lengths)[:-1]]).astype(np.int64)

    @property
    def key(self):
        return (tuple(self.slot_pad), self.t_pad)



def _copy(nc, eng, out, in_):
    # engine-dispatched copy: DVE has tensor_copy, ACT uses activation(Copy)
    if eng is nc.scalar:
        nc.scalar.copy(out, in_)
    else:
        eng.tensor_copy(out, in_)


def _transpose_qi(nc, at_ps, entry, L, nk):
    # attn^T @ diag(recip): transpose + normalize in one matmul per k-chunk
    attn, diag, lq, qoff = entry
    for ki in range(nk):
        kc = min(P, L - P * ki)
        nc.tensor.matmul(
            at_ps[ki][0:kc, qoff:qoff + lq],
            attn[0:lq, P * ki:P * ki + kc],
            diag[0:lq, 0:lq],
            start=True,
            stop=True,
        )

def _build_program(plan: Plan, loop_n: int = 1):
    """Build and compile the single-core Bass program (same for all cores).

    loop_n > 1 wraps the whole computation in a hardware For-loop (for
    steady-state timing measurements; the body is idempotent)."""
    import contextlib
    T = plan.t_pad
    NT = plan.nt
    nc = bacc.Bacc("TRN2", target_bir_lowering=False, debug=False)

    dt = mybir.dt
    # ---- DRAM I/O ----
    # xT packed per-slot: [p, s*(8*gw)] with [p, c, t] = x-dim c*128+p of
    # token t -> one contiguous 8KB descriptor per partition per slot
    xT_d = nc.dram_tensor("xT", [P, 8 * T], dt.bfloat16, kind="ExternalInput").ap()
    x_d = nc.dram_tensor("x", [T, D_MODEL], dt.float32, kind="ExternalInput").ap()
    # weights packed partition-major on host: one contiguous DMA each
    wq_d = nc.dram_tensor("wq", [P, 4 * 8 * P], dt.bfloat16, kind="ExternalInput").ap()
    wk_d = nc.dram_tensor("wk", [P, 4 * 8 * P], dt.bfloat16, kind="ExternalInput").ap()
    wv_d = nc.dram_tensor("wv", [P, 8 * D_HALF], dt.bfloat16, kind="ExternalInput").ap()
    pw_d = nc.dram_tensor("pw", [P, 8 * D_HALF], dt.bfloat16, kind="ExternalInput").ap()
    npad_d = nc.dram_tensor("npad", [P, 4], dt.float32, kind="ExternalInput").ap()
    ident_d = nc.dram_tensor("ident", [P, P], dt.bfloat16, kind="ExternalInput").ap()
    out_d = nc.dram_tensor("out", [T, D_MODEL], dt.float32, kind="ExternalOutput").ap()

    with tile.TileContext(nc) as tc:
        with (
            tc.tile_pool(name="persist", bufs=1) as pp,
            tc.tile_pool(name="weights", bufs=1) as wp,
        ):
            # Per-slot persistent tensors (finer dependency granularity lets
            # attention/proj start as soon as a slot's QKV is done).
            # Q^T/K^T pair-stacked: [p, comp(c/p), pair, region]; partition
            # p<64 holds head 2*pair, p>=64 head 2*pair+1 (comp's 64 dims).
            qt = [pp.tile([P, 2, 4, r], dt.bfloat16, name=f"qt{s}", tag=f"qt{s}")
                  for s, r in enumerate(plan.regions)]
            kt = [pp.tile([P, 2, 4, r], dt.bfloat16, name=f"kt{s}", tag=f"kt{s}")
                  for s, r in enumerate(plan.regions)]
            # V token-natural, head-major columns: [p, tile, head, {c64|p64}]
            vv = [pp.tile([P, r // P, D_MODEL], dt.bfloat16, name=f"vv{s}", tag=f"vv{s}")
                  for s, r in enumerate(plan.regions)]
            o1t = [pp.tile([P, 4, r], dt.bfloat16, name=f"o1t{s}", tag=f"o1t{s}")
                   for s, r in enumerate(plan.regions)]
            o2t = [pp.tile([P, 4, r], dt.bfloat16, name=f"o2t{s}", tag=f"o2t{s}")
                   for s, r in enumerate(plan.regions)]
            npad_sb = pp.tile([P, 4], dt.float32, tag="npad")
            ident_sb = pp.tile([P, P], dt.bfloat16, tag="ident")

            wq_sb = wp.tile([P, 4, 8, P], dt.bfloat16, tag="wq")
            wk_sb = wp.tile([P, 4, 8, P], dt.bfloat16, tag="wk")
            wv_sb = wp.tile([P, 8, D_HALF], dt.bfloat16, tag="wv")
            pw_sb = wp.tile([P, 2, 4, D_HALF], dt.bfloat16, tag="pw")

            _c = getattr(plan, "cfg", {})
            aux_eng = getattr(nc, _c.get("aux_eng", "gpsimd"))
            w_eng = getattr(nc, _c.get("w_eng", "scalar"))
            aux_eng.dma_start(npad_sb[:, :], npad_d[:, :])
            aux_eng.dma_start(ident_sb[:, :], ident_d[:, :])
            # per-pr-chunk weight DMAs on separate queues: the first QK
            # matmuls are gated on a 256KB chunk, not the full megabyte
            wq_f = wq_sb.rearrange("p a b c -> p a (b c)")
            wk_f = wk_sb.rearrange("p a b c -> p a (b c)")
            wq_r = wq_d.rearrange("p (a r) -> p a r", a=4)
            wk_r = wk_d.rearrange("p (a r) -> p a r", a=4)
            for pr in range(4):
                w_eng.dma_start(wq_f[:, pr, :], wq_r[:, pr, :])
                w_eng.dma_start(wk_f[:, pr, :], wk_r[:, pr, :])
            aux_eng.dma_start(
                wv_sb.rearrange("p a b -> p (a b)")[:, :], wv_d[:, :])
            aux_eng.dma_start(
                pw_sb.rearrange("p a b c -> p (a b c)")[:, :], pw_d[:, :])

            loop_cm = (tc.For_i(0, loop_n, 1,
                                hint_engines=(mybir.EngineType.PE,
                                              mybir.EngineType.DVE,
                                              mybir.EngineType.Activation,
                                              mybir.EngineType.SP))
                       if loop_n > 1 else contextlib.nullcontext())
            with loop_cm:
                _kernel_body(nc, tc, plan, locals())

    nc.compile()
    return nc


def _kernel_body(nc, tc, plan, env):
    dt = mybir.dt
    qt, kt, vv, o1t, o2t = (env["qt"], env["kt"], env["vv"], env["o1t"],
                            env["o2t"])
    npad_sb, ident_sb = env["npad_sb"], env["ident_sb"]
    wq_sb, wk_sb, wv_sb, pw_sb = (env["wq_sb"], env["wk_sb"], env["wv_sb"],
                                  env["pw_sb"])
    xT_d, x_d, out_d = env["xT_d"], env["x_d"], env["out_d"]
    cfg = getattr(plan, "cfg", dict(qk=4, v=3, lg=3, at=1, ot=1, z=6))
    out_eng = getattr(nc, cfg.get("out_eng", "gpsimd"))
    aux_eng = getattr(nc, cfg.get("aux_eng", "gpsimd"))
    w_eng = getattr(nc, cfg.get("w_eng", "scalar"))
    ms_gp = cfg.get("ms_gp", True)
    ln_gp = cfg.get("ln_gp", True)

    # zero the attention-output staging (pad-query columns are never
    # written; keep them finite for the projection matmuls)
    # only the pad columns [L, region) are never written by attention
    ms_eng = nc.gpsimd if ms_gp else nc.vector
    for s in range(4):
        L, r = plan.slot_pad[s], plan.regions[s]
        if L < r:
            ms_eng.memset(o1t[s][:, :, L:r], 0.0)
            ms_eng.memset(o2t[s][:, :, L:r], 0.0)

    # ================= Phase 1: QKV projections =================
    with (
        tc.tile_pool(name="xt_pool", bufs=2) as xtp,
        tc.tile_pool(name="qk_ps", bufs=cfg["qk"], space="PSUM") as qkps,
        tc.tile_pool(name="v_ps", bufs=cfg["v"], space="PSUM") as vps,
    ):
        for s in range(4):
            gw = plan.regions[s]
            g0 = plan.offs[s]
            xt_sb = xtp.tile([P, 8, gw], dt.bfloat16, tag="xt")
            nc.sync.dma_start(
                xt_sb.rearrange("p a b -> p (a b)")[:, :],
                xT_d[:, 8 * g0:8 * (g0 + gw)])
            for pr in range(4):
                for qk, (w_sb, dst) in enumerate(((wq_sb, qt), (wk_sb, kt))):
                    for half in range(2):  # 0: content dims, 1: pos dims
                        acc = qkps.tile([P, 512], dt.float32, tag="qkacc")
                        for jj in range(4):
                            j = half * 4 + jj
                            nc.tensor.matmul(
                                acc[:, 0:gw],
                                w_sb[:, pr, j, :],
                                xt_sb[:, j, 0:gw],
                                start=(jj == 0),
                                stop=(jj == 3),
                            )
                        eng = nc.vector if (pr + qk + half) % 2 else nc.scalar
                        _copy(nc, eng, dst[s][:, half, pr, 0:gw], acc[:, 0:gw])
            vv_w = vv[s].rearrange("p t (h b d) -> p t h b d", h=N_HEAD, b=2)
            for tt in range(gw // P):
                tl = tt * P
                for half in range(2):
                    vacc = vps.tile([P, 512], dt.float32, tag="vacc")
                    vacc_r = vacc.rearrange("p (h d) -> p h d", h=N_HEAD)
                    for jj in range(4):
                        j = half * 4 + jj
                        nc.tensor.matmul(
                            vacc[:, :],
                            xt_sb[:, j, tl:tl + P],
                            wv_sb[:, j, :],
                            start=(jj == 0),
                            stop=(jj == 3),
                        )
                    # scatter head h's 64 cols to h*128 + half*64
                    nc.scalar.copy(vv_w[:, tt, :, half, :], vacc_r[:, :, :])

    # ======== Phase 2+3: attention + proj/LN interleaved per slot ========
    # PSUM: lg(2) + atp0..3(4) + ozp(2, shared by attn-out and proj acc) = 8
    x_dma = nc.sync
    out_dma = out_eng
    with (
        tc.tile_pool(name="lg_ps", bufs=2, space="PSUM") as lgps,
        tc.tile_pool(name="at_ps", bufs=1, space="PSUM") as atps,
        tc.tile_pool(name="ozp_ps", bufs=2, space="PSUM") as ozps,
        tc.tile_pool(name="attn_sb", bufs=4) as asb,
        tc.tile_pool(name="small_sb", bufs=6) as ssb,
        tc.tile_pool(name="z_sb", bufs=3) as zsb,
        tc.tile_pool(name="x_sb", bufs=3) as xsb,
        tc.tile_pool(name="ln_sb", bufs=4) as lsb,
    ):
        def _slot_dims(s):
            L = plan.slot_pad[s]
            nk = (L + P - 1) // P
            return L, nk

        def _emit_p3(s):
            # proj + residual + LN for slot s's token tiles
            gw = plan.regions[s]
            g0 = plan.offs[s]
            for tt in range(gw // P):
                lt = tt * P
                t0 = g0 + lt
                zh = []
                for i, osrc in enumerate((o1t[s], o2t[s])):
                    zp = ozps.tile([P, 512], dt.float32, name="zp", tag="ozp")
                    for k in range(4):
                        nc.tensor.matmul(
                            zp[:, :],
                            osrc[:, k, lt:lt + P],
                            pw_sb[:, i, k, :],
                            start=(k == 0),
                            stop=(k == 3),
                        )
                    zh.append(zp)
                xt_f = xsb.tile([P, D_MODEL], dt.float32, tag="xf")
                x_dma.dma_start(xt_f[:, :], x_d[t0:t0 + P, :])
                z = zsb.tile([P, D_MODEL], dt.float32, tag="z")
                zsum = lsb.tile([P, 1], dt.float32, tag="zsum")
                ssq = lsb.tile([P, 1], dt.float32, tag="ssq")
                for i in range(2):
                    nc.vector.tensor_tensor(
                        z[:, i * D_HALF:(i + 1) * D_HALF],
                        zh[i][:, :],
                        xt_f[:, i * D_HALF:(i + 1) * D_HALF],
                        mybir.AluOpType.add,
                    )
                nc.vector.reduce_sum(
                    zsum[:, :], z[:, :], axis=mybir.AxisListType.X
                )
                nc.scalar.activation(
                    xt_f[:, :], z[:, :],
                    mybir.ActivationFunctionType.Square,
                    accum_out=ssq[:, :],
                )
                mu = lsb.tile([P, 1], dt.float32, tag="mu")
                nc.vector.tensor_scalar(
                    mu[:, :], zsum[:, :], 1.0 / D_MODEL, None,
                    mybir.AluOpType.mult,
                )
                var = lsb.tile([P, 1], dt.float32, tag="var")
                nc.vector.tensor_tensor(
                    var[:, :], zsum[:, :], mu[:, :], mybir.AluOpType.mult
                )
                nc.vector.tensor_tensor(
                    var[:, :], ssq[:, :], var[:, :], mybir.AluOpType.subtract
                )
                sig = lsb.tile([P, 1], dt.float32, tag="sig")
                nc.scalar.activation(
                    sig[:, :], var[:, :], mybir.ActivationFunctionType.Sqrt,
                    scale=1.0 / (D_MODEL - 1),
                )
                nc.vector.tensor_scalar(
                    sig[:, :], sig[:, :], EPS, None, mybir.AluOpType.add
                )
                rstd = lsb.tile([P, 1], dt.float32, tag="rstd")
                nc.vector.reciprocal(rstd[:, :], sig[:, :])
                negmu = lsb.tile([P, 1], dt.float32, tag="negmu")
                nc.vector.tensor_scalar(
                    negmu[:, :], zsum[:, :], -1.0 / D_MODEL, None,
                    mybir.AluOpType.mult,
                )
                o = zsb.tile([P, D_MODEL], dt.float32, tag="o")
                eng_o = nc.gpsimd if (ln_gp and tt % 2 == 0) else nc.vector
                eng_o.tensor_scalar(
                    o[:, :], z[:, :], negmu[:, :], rstd[:, :],
                    mybir.AluOpType.add, mybir.AluOpType.mult,
                )
                out_dma.dma_start(out_d[t0:t0 + P, :], o[:, :])

        def _stage_prev(prev):
            # stage prev head's normalized attn^T slabs PSUM->SBUF
            ps, pot, pat, ph = prev
            L, nk = _slot_dims(ps)
            sbs = []
            for ki in range(nk):
                kc = min(P, L - P * ki)
                at_sb = asb.tile([P, 512], dt.bfloat16, tag="at_sb")
                eng = nc.vector if (ki % 2 == 0) else nc.scalar
                _copy(nc, eng, at_sb[0:kc, 0:L], pat[ki][0:kc, 0:L])
                sbs.append((at_sb, kc))
            return sbs

        def _prev_ot(prev, sbs, ki):
            # one attn@V chunk of the prev head, streamed under the
            # current head's logits
            ps, pot, pat, ph = prev
            L, nk = _slot_dims(ps)
            pvv = vv[ps].rearrange("p t (h d) -> p t h d", h=N_HEAD)
            sb, kc = sbs[ki]
            nc.tensor.matmul(
                pot[:, 0:L],
                pvv[0:kc, ki, ph, :],
                sb[0:kc, 0:L],
                start=(ki == 0),
                stop=(ki == nk - 1),
            )

        def _prev_out(prev):
            ps, pot, pat, ph = prev
            L, nk = _slot_dims(ps)
            php, ppr = 64 * (ph % 2), ph // 2
            nc.vector.tensor_copy(
                o1t[ps][php:php + 64, ppr, 0:L], pot[0:64, 0:L])
            nc.vector.tensor_copy(
                o2t[ps][php:php + 64, ppr, 0:L], pot[64:128, 0:L])

        prev = None       # (slot, ot_psum, at_ps, head) not yet V-multiplied
        pend_p3 = None    # slot whose proj/LN is not yet emitted
        for s in range(4):
            L = plan.slot_pad[s]
            nk = (L + P - 1) // P
            nq = nk
            for h in range(N_HEAD):
                half, pr = h % 2, h // 2
                hp = 64 * half
                # attn^T slabs per key-chunk: [kc, all slot queries] so the
                # attn@V contraction runs one N=L matmul per chunk
                at_ps = [atps.tile([P, 512], dt.float32, name=f"atp{ki}",
                                   tag=f"atp{ki}") for ki in range(nk)]
                ot = ozps.tile([P, 512], dt.float32, name="ot", tag="ozp")
                sbs = _stage_prev(prev) if prev is not None else None
                pnk = _slot_dims(prev[0])[1] if prev is not None else 0
                pend = {}
                for qi in range(nq):
                    qoff = P * qi
                    lq = min(P, L - P * qi)
                    lg = lgps.tile([P, 512], dt.float32, tag="lg")
                    for comp in range(2):
                        nc.tensor.matmul(
                            lg[0:lq, 0:L],
                            qt[s][hp:hp + 64, comp, pr, qoff:qoff + lq],
                            kt[s][hp:hp + 64, comp, pr, 0:L],
                            start=(comp == 0),
                            stop=(comp == 1),
                        )
                    if prev is not None and qi < pnk:
                        _prev_ot(prev, sbs, qi)
                    attn = asb.tile([P, 512], dt.bfloat16, tag="attn")
                    se = ssb.tile([P, 1], dt.float32, tag="se")
                    nc.scalar.activation(
                        attn[0:lq, 0:L],
                        lg[0:lq, 0:L],
                        mybir.ActivationFunctionType.Exp,
                        scale=1.0 / SCALE,
                        accum_out=se[0:lq, :],
                    )
                    rc = ssb.tile([P, 1], dt.float32, tag="rc")
                    nc.vector.tensor_tensor(
                        rc[0:lq, :], se[0:lq, :], npad_sb[0:lq, s:s + 1],
                        mybir.AluOpType.subtract,
                    )
                    nc.vector.reciprocal(rc[0:lq, :], rc[0:lq, :])
                    diag = ssb.tile([P, P], dt.bfloat16, tag="diag")
                    nc.vector.tensor_scalar(
                        diag[0:lq, 0:lq], ident_sb[0:lq, 0:lq],
                        rc[0:lq, :], None, mybir.AluOpType.mult,
                    )
                    # previous qi's transpose runs after this qi's logits so
                    # exp/diag have a full lg of slack
                    if qi >= 1:
                        _transpose_qi(nc, at_ps, pend[qi - 1], L, nk)
                    pend[qi] = (attn, diag, lq, qoff)
                # flush: prev head's remaining ot chunks, last transpose
                if prev is not None:
                    for ki in range(nq, pnk):
                        _prev_ot(prev, sbs, ki)
                _transpose_qi(nc, at_ps, pend[nq - 1], L, nk)
                if prev is not None:
                    _prev_out(prev)
                prev = (s, ot, at_ps, h)
                if h == 0 and pend_p3 is not None:
                    _emit_p3(pend_p3)
                    pend_p3 = None
            pend_p3 = s
        # drain the final head and the last slot's proj/LN
        sbs = _stage_prev(prev)
        for ki in range(nk):
            _prev_ot(prev, sbs, ki)
        _prev_out(prev)
        _emit_p3(3)


_PROGRAMS = {}   # plan.key -> (nc, plan)
_RUNNERS = {}    # plan.key -> callable(in_maps) -> list[dict]


def _get_program(plan: Plan):
    if plan.key not in _PROGRAMS:
        _PROGRAMS[plan.key] = _build_program(plan)
    return _PROGRAMS[plan.key]


def _make_runner(nc, donate=True):
    """Cached PJRT runner (mirrors bass_utils.run_bass_kernel_spmd's axon
    path via bass2jax, but reuses the jitted executable across calls)."""
    import jax
    from jax.sharding import Mesh, PartitionSpec
    from jax.experimental.shard_map import shard_map
    from concourse import bass2jax

    bass2jax.install_neuronx_cc_hook()

    partition_name = (nc.partition_id_tensor.name
                      if nc.partition_id_tensor else None)
    in_names, out_names, out_avals, zero_shapes = [], [], [], []
    for alloc in nc.m.functions[0].allocations:
        if not isinstance(alloc, mybir.MemoryLocationSet):
            continue
        name = alloc.memorylocations[0].name
        if alloc.kind == "ExternalInput":
            if name == partition_name:
                continue
            in_names.append(name)
        elif alloc.kind == "ExternalOutput":
            out_names.append(name)
            shape = tuple(alloc.tensor_shape)
            dtype = mybir.dt.np(alloc.dtype)
            out_avals.append(jax.core.ShapedArray(shape, dtype))
            zero_shapes.append((shape, dtype))
    n_params = len(in_names)
    all_names = in_names + out_names
    if partition_name is not None:
        all_names = all_names + [partition_name]

    def _body(*args):
        operands = list(args)
        if partition_name is not None:
            operands.append(bass2jax.partition_id_tensor())
        outs = bass2jax._bass_exec_p.bind(
            *operands,
            out_avals=tuple(out_avals),
            in_names=tuple(all_names),
            out_names=tuple(out_names),
            lowering_input_output_aliases=(),
            sim_require_finite=True,
            sim_require_nnan=True,
            nc=nc,
        )
        return tuple(outs)

    devices = jax.devices()[:N_CORES]
    mesh = Mesh(np.asarray(devices), ("core",))
    in_specs = (PartitionSpec("core"),) * (n_params + len(out_names))
    out_specs = (PartitionSpec("core"),) * len(out_names)
    sharded = jax.jit(
        shard_map(_body, mesh=mesh, in_specs=in_specs, out_specs=out_specs,
                  check_rep=False),
        donate_argnums=tuple(range(n_params, n_params + len(out_names)))
        if donate else (),
        keep_unused=True,
    )

    def run(in_maps):
        concat_in = [
            np.concatenate([np.asarray(m[name]) for m in in_maps], axis=0)
            for name in in_names
        ]
        concat_zeros = [
            np.zeros((N_CORES * s[0], *s[1:]), d) for (s, d) in zero_shapes
        ]
        out_arrs = sharded(*concat_in, *concat_zeros)
        return [
            {
                name: np.asarray(out_arrs[i]).reshape(
                    N_CORES, *out_avals[i].shape)[c]
                for i, name in enumerate(out_names)
            }
            for c in range(N_CORES)
        ]

    run.sharded = sharded
    run.in_names = in_names
    run.out_names = out_names
    run.out_avals = out_avals
    run.zero_shapes = zero_shapes
    run.n_params = n_params
    return run


def _prep_weights(w_qs1, w_ks1, w_vs1, w_qs2, w_ks2, w_vs2, proj1_w, proj2_w):
    wq, wk, wv, pw = _prep_weights_4d(w_qs1, w_ks1, w_vs1, w_qs2, w_ks2,
                                      w_vs2, proj1_w, proj2_w)
    # partition-major packing: one contiguous DMA per weight tensor on device
    wq = np.ascontiguousarray(wq.transpose(2, 0, 1, 3).reshape(P, -1))
    wk = np.ascontiguousarray(wk.transpose(2, 0, 1, 3).reshape(P, -1))
    wv = np.ascontiguousarray(wv.transpose(1, 0, 2).reshape(P, -1))
    pw = np.ascontiguousarray(pw.transpose(2, 0, 1, 3).reshape(P, -1))
    return wq, wk, wv, pw


def _prep_weights_4d(w_qs1, w_ks1, w_vs1, w_qs2, w_ks2, w_vs2, proj1_w, proj2_w):
    wq = np.zeros((4, 8, P, P), BF16)
    wk = np.zeros((4, 8, P, P), BF16)
    for pr in range(4):
        h0, h1 = 2 * pr, 2 * pr + 1
        for j in range(8):
            if j < 4:
                rows = slice(j * P, (j + 1) * P)
                wq[pr, j] = np.concatenate(
                    [w_qs1[h0, rows, :], w_qs1[h1, rows, :]], axis=1).astype(BF16)
                wk[pr, j] = np.concatenate(
                    [w_ks1[h0, rows, :], w_ks1[h1, rows, :]], axis=1).astype(BF16)
            else:
                rows = slice((j - 4) * P, (j - 3) * P)
                wq[pr, j] = np.concatenate(
                    [w_qs2[h0, rows, :], w_qs2[h1, rows, :]], axis=1).astype(BF16)
                wk[pr, j] = np.concatenate(
                    [w_ks2[h0, rows, :], w_ks2[h1, rows, :]], axis=1).astype(BF16)
    wv = np.zeros((8, P, D_HALF), BF16)
    for j in range(8):
        src = w_vs1 if j < 4 else w_vs2
        rows = slice((j % 4) * P, (j % 4 + 1) * P)
        wv[j] = np.concatenate([src[h, rows, :] for h in range(8)], axis=1
                               ).astype(BF16)
    pw = np.zeros((2, 4, P, D_HALF), BF16)
    p1T = np.ascontiguousarray(proj1_w.T)  # [in, out]
    p2T = np.ascontiguousarray(proj2_w.T)
    for k in range(4):
        pw[0, k] = p1T[k * P:(k + 1) * P, :].astype(BF16)
        pw[1, k] = p2T[k * P:(k + 1) * P, :].astype(BF16)
    return wq, wk, wv, pw


def _prep_core_inputs(plan: Plan, inp, c):
    T = plan.t_pad
    x = np.zeros((T, D_MODEL), F32)
    npad = np.zeros((4,), F32)
    for j in range(4):
        s = plan.core_sents[c][j]
        L = int(plan.lengths[s])
        g0 = int(plan.glob_off[s])
        x[plan.offs[j]:plan.offs[j] + L] = inp[g0:g0 + L]
        npad[j] = plan.slot_pad[j] - L
    # per-slot packed transpose: [p, c, t] = x[t, c*128+p], slots contiguous
    xT = np.zeros((P, 8 * T), BF16)
    for j in range(4):
        gw, g0 = plan.regions[j], plan.offs[j]
        blk = x[g0:g0 + gw].T.reshape(8, P, gw).transpose(1, 0, 2)
        xT[:, 8 * g0:8 * (g0 + gw)] = blk.reshape(P, 8 * gw).astype(BF16)
    npad_rep = np.tile(npad[None, :], (P, 1)).astype(F32)
    return x, xT, npad_rep


def make_in_maps(plan: Plan, inp, weights):
    wq, wk, wv, pw = weights
    ident = np.eye(P, dtype=BF16)
    in_maps = []
    for c in range(N_CORES):
        x, xT, npad_rep = _prep_core_inputs(plan, inp, c)
        in_maps.append({
            "xT": xT, "x": x, "wq": wq, "wk": wk, "wv": wv, "pw": pw,
            "npad": npad_rep, "ident": ident,
        })
    return in_maps


def gather_output(plan: Plan, results, a_2=None, b_2=None):
    T_tot = int(plan.lengths.sum())
    out = np.empty((T_tot, D_MODEL), F32)
    for c in range(N_CORES):
        oc = results[c]["out"]
        for j in range(4):
            s = plan.core_sents[c][j]
            L = int(plan.lengths[s])
            g0 = int(plan.glob_off[s])
            out[g0:g0 + L] = oc[plan.offs[j]:plan.offs[j] + L]
    if a_2 is not None and (np.any(a_2 != 1.0) or np.any(b_2 != 0.0)):
        out = out * np.asarray(a_2, F32) + np.asarray(b_2, F32)
    return out


def kernel(inp, w_qs1, w_ks1, w_vs1, w_qs2, w_ks2, w_vs2,
           proj1_w, proj2_w, a_2, b_2, token_batch, token_pos, valid_mask):
    inp = np.asarray(inp, F32)
    token_batch = np.asarray(token_batch)
    lengths = np.bincount(token_batch, minlength=MB).astype(np.int64)
    # tokens of each sentence must be contiguous and in order
    plan = Plan(lengths)

    nc = _get_program(plan)
    if plan.key not in _RUNNERS:
        _RUNNERS[plan.key] = _make_runner(nc)
    runner = _RUNNERS[plan.key]

    weights = _prep_weights(np.asarray(w_qs1), np.asarray(w_ks1),
                            np.asarray(w_vs1), np.asarray(w_qs2),
                            np.asarray(w_ks2), np.asarray(w_vs2),
                            np.asarray(proj1_w), np.asarray(proj2_w))
    in_maps = make_in_maps(plan, inp, weights)
    results = runner(in_maps)
    return gather_output(plan, results, np.asarray(a_2), np.asarray(b_2))

